# revision 8
# baseline (speedup 1.0000x reference)
"""Trainium2 Bass kernel for top-1 MoE expert MLP (nn_Experts problem).

Strategy (expert-parallel, one expert per NeuronCore):
  - Routing is one-hot top-1: each token is processed by exactly one expert,
    so each core computes the MLP only for the tokens routed to its expert.
  - Host-side shard step: compute token->expert assignment from
    dispatch_tensor, gather each expert's tokens (transposed to [D, CAP]),
    and pack w1 into per-m1 column blocks so every DMA is contiguous.
  - Device phase A: h^T[F,CAP] = gelu(w1^T @ x^T + b1)   (w1 stationary)
  - Device phase B: y^T[D,CAP] via matmul(yT_tile,
        lhsT=w2[F-tile, D-tile] (native layout), rhs=h^T)
    so w2 needs no host repacking and the output has no token-tile padding.
  - Host-side unshard: scatter per-expert columns back to token order,
    apply the combine gate and shared bias b2.
  - CAP = 512 tokens/expert (capacity factor 1.0, the standard MoE expert
    capacity).  Tokens beyond CAP for an over-subscribed expert take the
    exact fp32/erf host fallback path, so the result is correct for any
    routing distribution.

Perf notes (from NTFF traces):
  - DMA descriptor generation costs ~3ns/row serially on the issuing
    engine, so the input DMAs are spread across all five engine queues;
    descgen then runs concurrently and the early tiles land ~4us sooner.
  - A few dummy matmuls on a memset tile run while the first DMA is in
    flight so the PE HAM clock-gate (1.2 GHz cold -> 2.4 GHz after ~3.4us
    of sustained busy) starts warming before real data arrives.
  - The Bacc preamble/teardown (full 256-semaphore clear chains) is a
    fixed ~8us cost outside kernel control.
"""

import numpy as np

B, N, D, E, F = 8, 512, 1024, 8, 2048
T = B * N
P = 128
CAP = 512            # per-expert token capacity (capacity factor 1.0)
KT1 = D // P         # 8  k-tiles for matmul1 (contract over D)
MT1 = F // P         # 16 m-tiles for matmul1 / k-tiles for matmul2
MT2 = D // P         # 8  m-tiles for matmul2 (output yT partition tiles)
MM_DT = "bfloat16"   # matmul dtype: "bfloat16" (fast) or "float32r"
N_WARM = 6           # PE warm-up dummy matmuls (run while first DMA lands)

# token chunks per PSUM tile (one matmul output <= 512 fp32 = one bank)
if CAP <= 512:
    CHUNKS = ((0, CAP),)
else:
    CHUNKS = ((0, CAP // 2), (CAP // 2, CAP))

# w1 m1-tile DMA block sizes for m1 = 1..15 (m1=0 travels in `head`):
# fine-grained at the front so early matmuls aren't gated, coarser later.
W1_BLOCKS = (1, 2, 4, 4, 4)

_NC_CACHE = {}


def _round_fp32r(a):
    """Round-to-nearest-even at mantissa bit 12 (fp32r keeps the top 20 bits
    of an fp32 word: 1 sign + 8 exp + 11 explicit mantissa bits)."""
    u = a.view(np.uint32)
    lsb = (u >> 12) & 1
    u = u + 0x7FF + lsb
    u &= np.uint32(0xFFFFF000)
    return u.view(np.float32)


def _build_bass():
    import concourse.bacc as bacc
    import concourse.tile as tile
    from concourse import mybir

    f32 = mybir.dt.float32
    mm_dt = getattr(mybir.dt, MM_DT)

    nc = bacc.Bacc(None, target_bir_lowering=False)
    # head packs the xT k=0 block together with the full w1 m1=0 row-block;
    # its first slice (x k0 + w1 m0k0) unblocks the very first matmul.
    head = nc.declare_dram_parameter("head", [P, CAP + D], mm_dt,
                                     isOutput=False)
    xr = nc.declare_dram_parameter("xr", [(KT1 - 1) * P, CAP], mm_dt,
                                   isOutput=False)
    gb = nc.declare_dram_parameter("gb", [P, MT1], f32, isOutput=False)
    w1r = nc.declare_dram_parameter("w1r", [(MT1 - 1) * P, D], mm_dt,
                                    isOutput=False)
    w2s = nc.declare_dram_parameter("w2s", [F, D], mm_dt, isOutput=False)
    y = nc.declare_dram_parameter("y", [D, CAP], mm_dt, isOutput=True)

    w1_off = []
    off = 1
    for nm in W1_BLOCKS:
        w1_off.append(off)
        off += nm
    assert off == MT1

    with tile.TileContext(nc) as tc:
        with (
            tc.tile_pool(name="wrm", bufs=1) as wrm,
            tc.tile_pool(name="hdp", bufs=1) as hdp,
            tc.tile_pool(name="xp", bufs=1) as xp,
            tc.tile_pool(name="gbp", bufs=1) as gbp,
            tc.tile_pool(name="w1p", bufs=len(W1_BLOCKS)) as w1p,
            tc.tile_pool(name="w2p", bufs=2) as w2p,
            tc.tile_pool(name="hp", bufs=MT1) as hp,
            tc.tile_pool(name="stp", bufs=2) as stp,
            tc.tile_pool(name="psA", bufs=4, space="PSUM") as psA,
            tc.tile_pool(name="psB", bufs=4, space="PSUM") as psB,
        ):
            # PE warm-up: memset a small tile, then issue dummy matmuls that
            # keep the PE busy (heating the HAM clock-gate) while the first
            # real DMA is still in flight.
            garb = wrm.tile([P, 256], mm_dt, tag="garb")
            nc.gpsimd.memset(garb[:], 0)
            for wi in range(N_WARM):
                dummy = psB.tile([P, CHUNKS[0][1] - CHUNKS[0][0]], f32,
                                 tag="psB0", name=f"warm_{wi}")
                nc.tensor.matmul(dummy[:2, :256], garb[:, 0:2], garb[:],
                                 start=True, stop=True, skip_group_check=True)

            # Input DMAs are spread across the three DMA-capable engine
            # queues (sync/gpsimd/scalar) so descriptor generation (~3ns/row,
            # serial per engine) runs concurrently:
            #   sync:   head_a, w1 blocks (ordered m1 stream), y outs
            #   gpsimd: head_b, x k=1..3, gb, w2 lower half
            #   scalar: x k=4..7, w2 upper half
            HA = CAP + P
            head_a = hdp.tile([P, HA], mm_dt, tag="ha")
            nc.sync.dma_start(out=head_a[:], in_=head[:, 0:HA])
            head_b = hdp.tile([P, D - P], mm_dt, tag="hb")
            nc.gpsimd.dma_start(out=head_b[:], in_=head[:, HA:])

            XSPLIT = 4  # k = 1..3 with the x-lo DMA, 4..7 with x-hi
            x_sb = []
            for eng, (k0, k1) in ((nc.gpsimd, (1, XSPLIT)),
                                  (nc.scalar, (XSPLIT, KT1))):
                t = xp.tile([P, k1 - k0, CAP], mm_dt, tag=f"x{k0}",
                            name=f"x_{k0}")
                src = xr[(k0 - 1) * P:(k1 - 1) * P, :].rearrange(
                    "(j p) c -> p j c", p=P)
                eng.dma_start(out=t[:], in_=src)
                x_sb.append((k0, t))
            gb_sb = gbp.tile([P, MT1], f32)
            nc.gpsimd.dma_start(out=gb_sb[:], in_=gb[:, :])

            w1_blk = []
            for j, nm in enumerate(W1_BLOCKS):
                t = w1p.tile([P, nm, D], mm_dt, tag="w1", name=f"w1_{j}",
                             padded_shape=[P, max(W1_BLOCKS), D])
                r0 = (w1_off[j] - 1) * P
                src = w1r[r0:r0 + nm * P, :].rearrange(
                    "(j p) d -> p j d", p=P)
                nc.sync.dma_start(out=t[:], in_=src)
                w1_blk.append(t)

            w2_sb = []
            for j, eng in enumerate((nc.gpsimd, nc.scalar)):
                t = w2p.tile([P, MT1 // 2, D], mm_dt, tag="w2",
                             name=f"w2_{j}")
                src = w2s[j * (F // 2):(j + 1) * (F // 2), :].rearrange(
                    "(j p) d -> p j d", p=P)
                eng.dma_start(out=t[:], in_=src)
                w2_sb.append(t)

            def x_rhs(k, a, b):
                if k == 0:
                    return head_a[:, a:b]
                for k0, t in reversed(x_sb):
                    if k >= k0:
                        return t[:, k - k0, a:b]
                raise AssertionError

            def w1_lhs(m1, k):
                if m1 == 0:
                    if k == 0:
                        return head_a[:, CAP:CAP + P]
                    return head_b[:, (k - 1) * P:k * P]
                j = next(i for i in range(len(W1_BLOCKS))
                         if w1_off[i] <= m1 < w1_off[i] + W1_BLOCKS[i])
                return w1_blk[j][:, m1 - w1_off[j], k * P:(k + 1) * P]

            def primer(name, lhs1, rhs1):
                # borrows an (idle until phase B) psB slot; touches a fresh
                # weight block on PE so later matmuls stay single-wait
                dummy = psB.tile([P, CHUNKS[0][1] - CHUNKS[0][0]], f32,
                                 tag="psB0", name=f"prime_{name}")
                nc.tensor.matmul(dummy[:2, :256], lhs1, rhs1,
                                 start=True, stop=True, skip_group_check=True)

            # Phase A: h^T[F, CAP] = gelu(w1^T @ x^T + b1)
            gelu = mybir.ActivationFunctionType.Gelu
            h_sb = []
            for m1 in range(MT1):
                if m1 > 0 and m1 in w1_off:
                    blk = w1_blk[w1_off.index(m1)]
                    primer(f"w1_{m1}", blk[:, 0, 0:2], blk[:, 0, 0:256])
                pss = [psA.tile([P, b - a], f32, tag=f"psA{i}",
                                name=f"psA{i}_{m1}")
                       for i, (a, b) in enumerate(CHUNKS)]
                for k in range(KT1):
                    lhs = w1_lhs(m1, k)
                    st, sp = (k == 0), (k == KT1 - 1)
                    for i, (a, b) in enumerate(CHUNKS):
                        nc.tensor.matmul(pss[i][:], lhs, x_rhs(k, a, b),
                                         start=st, stop=sp)
                h = hp.tile([P, CAP], mm_dt, tag="h", name=f"h_{m1}")
                bias = gb_sb[:, m1:m1 + 1]
                for i, (a, b) in enumerate(CHUNKS):
                    nc.scalar.activation(h[:, a:b], pss[i][:], gelu, bias=bias)
                h_sb.append(h)

            # Phase B: y^T[D, CAP]; w2 (native [F, D] layout) is the
            # stationary operand, h^T streams.  Output partition dim = D has
            # no padding; the gate moves to the host-side combine.
            # Output pairs (m2 0..5) ship two tiles per DMA; the last two m2
            # ship singly so the critical tail transfer is small.
            for j in range(2):
                primer(f"w2_{j}", w2_sb[j][:, 0, 0:2], w2_sb[j][:, 0, 0:256])
            cp = mybir.ActivationFunctionType.Copy
            stage = None
            for m2 in range(MT2):
                pss = [psB.tile([P, b - a], f32, tag=f"psB{i}",
                                name=f"psB{i}_{m2}")
                       for i, (a, b) in enumerate(CHUNKS)]
                for k2 in range(MT1):
                    lhs = w2_sb[k2 // (MT1 // 2)][:, k2 % (MT1 // 2),
                                                  m2 * P:(m2 + 1) * P]
                    st, sp = (k2 == 0), (k2 == MT1 - 1)
                    for i, (a, b) in enumerate(CHUNKS):
                        nc.tensor.matmul(pss[i][:], lhs,
                                         h_sb[k2][:, a:b], start=st, stop=sp)
                def copy_ps(dst, src, use_vec):
                    if use_vec:
                        nc.vector.tensor_scalar_add(dst, src, 0.0)
                    else:
                        nc.scalar.activation(dst, src, cp)

                if m2 < MT2 - 2:
                    half = m2 % 2
                    if half == 0:
                        stage = stp.tile([P, 2, CAP], mm_dt, tag="stage",
                                         name=f"stage_{m2 // 2}")
                    for i, (a, b) in enumerate(CHUNKS):
                        copy_ps(stage[:, half, a:b], pss[i][:], m2 % 2 == 1)
                    if half == 1:
                        dst = y[(m2 - 1) * P:(m2 + 1) * P, :].rearrange(
                            "(j p) c -> p j c", p=P)
                        nc.sync.dma_start(out=dst, in_=stage[:])
                else:
                    # last two tiles: copy split across scalar+vector, then a
                    # small single-tile DMA each, to shorten the drain tail
                    st1 = stp.tile([P, 1, CAP], mm_dt, tag="stage",
                                   name=f"stage_s{m2}")
                    mid = CAP // 2
                    nc.scalar.activation(st1[:, 0, 0:mid],
                                         pss[0][:, 0:mid], cp)
                    nc.vector.tensor_scalar_add(st1[:, 0, mid:CAP],
                                                pss[-1][:, mid - CHUNKS[-1][0]:],
                                                0.0)
                    nc.sync.dma_start(out=y[m2 * P:(m2 + 1) * P, :],
                                      in_=st1[:, 0, :])
    if not nc.is_finalized():
        nc.finalize()
    return nc


def _get_nc():
    if "nc" not in _NC_CACHE:
        _NC_CACHE["nc"] = _build_bass()
    return _NC_CACHE["nc"]


def kernel(x, dispatch_tensor, combine_tensor, w1, b1, w2, b2, **_):
    import ml_dtypes
    from concourse.bass_utils import run_bass_kernel_spmd

    mm_np = ml_dtypes.bfloat16 if MM_DT == "bfloat16" else np.float32
    rnd = (lambda a: a.astype(mm_np)) if MM_DT == "bfloat16" else _round_fp32r

    x = np.ascontiguousarray(np.asarray(x, dtype=np.float32)).reshape(T, D)
    dispatch = np.asarray(dispatch_tensor, dtype=np.float32).reshape(T, E)
    combine = np.asarray(combine_tensor, dtype=np.float32).reshape(T, E)
    w1 = np.asarray(w1, dtype=np.float32)
    b1 = np.asarray(b1, dtype=np.float32)
    w2 = np.asarray(w2, dtype=np.float32)
    b2 = np.asarray(b2, dtype=np.float32)

    top = dispatch.argmax(-1)
    gate = combine.sum(-1)
    full = [np.nonzero(top == e)[0] for e in range(E)]
    idxs = [idx[:CAP] for idx in full]
    spill = [idx[CAP:] for idx in full]  # over-capacity tokens -> host path

    in_maps = []
    for e in range(E):
        idx = idxs[e]
        c = len(idx)
        xT = np.zeros((D, CAP), np.float32)
        xT[:, :c] = x[idx].T
        # w1s[m1*P+p, k*P+m] = w1[k*P+p, m1*P+m]: per-m1 [P, D] row blocks
        # whose [:, k*P:(k+1)*P] slice is the lhsT k-tile for output tile m1.
        w1s = np.ascontiguousarray(
            w1[e].reshape(KT1, P, MT1, P).transpose(2, 1, 0, 3)
        ).reshape(F, D)
        gbm = np.ascontiguousarray(b1[e].reshape(MT1, P).T)
        xTr, w1sr = rnd(xT), rnd(w1s)
        in_maps.append({
            "head": np.ascontiguousarray(
                np.concatenate([xTr[:P], w1sr[:P]], axis=1)),
            "xr": np.ascontiguousarray(xTr[P:]),
            "gb": gbm,
            "w1r": np.ascontiguousarray(w1sr[P:]),
            "w2s": rnd(np.ascontiguousarray(w2[e])),
        })

    global _LAST_IN_MAPS
    _LAST_IN_MAPS = in_maps
    nc = _get_nc()
    res = run_bass_kernel_spmd(nc, in_maps, list(range(E)))

    y_flat = np.empty((T, D), np.float32)
    for e in range(E):
        idx = idxs[e]
        c = len(idx)
        yT = np.asarray(res.results[e]["y"]).astype(np.float32)  # [D, CAP]
        y_flat[idx] = yT[:, :c].T * gate[idx][:, None]
        if len(spill[e]):
            # capacity-overflow fallback (exact fp32 math on host)
            import math

            erf = np.frompyfunc(math.erf, 1, 1)
            hs = x[spill[e]] @ w1[e] + b1[e]
            hs = hs * 0.5 * (1.0 + erf(hs / np.sqrt(2.0)).astype(np.float64))
            y_flat[spill[e]] = (hs.astype(np.float32) @ w2[e]) * \
                gate[spill[e]][:, None]
    return (y_flat + b2[None, :]).reshape(B, N, D)


# revision 9
# speedup vs baseline: 1.1906x; 1.1906x over previous
"""Trainium2 Bass kernel for top-1 MoE expert MLP (nn_Experts problem).

Strategy (expert-parallel, one expert per NeuronCore):
  - Routing is one-hot top-1: each token is processed by exactly one expert,
    so each core computes the MLP only for the tokens routed to its expert.
  - Host-side shard step: compute token->expert assignment from
    dispatch_tensor, gather each expert's tokens (transposed to [D, CAP]),
    and pack w1 into per-m1 column blocks so every DMA is contiguous.
  - Device phase A: h^T[F,CAP] = gelu(w1^T @ x^T + b1)   (w1 stationary)
  - Device phase B: y^T[D,CAP] via matmul(yT_tile,
        lhsT=w2[F-tile, D-tile] (native layout), rhs=h^T)
    so w2 needs no host repacking and the output has no token-tile padding.
  - Host-side unshard: scatter per-expert columns back to token order,
    apply the combine gate and shared bias b2.
  - CAP = 512 tokens/expert (capacity factor 1.0, the standard MoE expert
    capacity).  Tokens beyond CAP for an over-subscribed expert take the
    exact fp32/erf host fallback path, so the result is correct for any
    routing distribution.

Perf notes (from NTFF traces):
  - DMA descriptor generation costs ~3ns/row serially on the issuing
    engine, so the input DMAs are spread across all five engine queues;
    descgen then runs concurrently and the early tiles land ~4us sooner.
  - A few dummy matmuls on a memset tile run while the first DMA is in
    flight so the PE HAM clock-gate (1.2 GHz cold -> 2.4 GHz after ~3.4us
    of sustained busy) starts warming before real data arrives.
  - The Bacc preamble/teardown (full 256-semaphore clear chains) is a
    fixed ~8us cost outside kernel control.
"""

import numpy as np

B, N, D, E, F = 8, 512, 1024, 8, 2048
T = B * N
P = 128
CAP = 512            # per-expert token capacity (capacity factor 1.0)
KT1 = D // P         # 8  k-tiles for matmul1 (contract over D)
MT1 = F // P         # 16 m-tiles for matmul1 / k-tiles for matmul2
MT2 = D // P         # 8  m-tiles for matmul2 (output yT partition tiles)
MM_DT = "bfloat16"   # matmul dtype: "bfloat16" (fast) or "float32r"
N_WARM = 6           # PE warm-up dummy matmuls (run while first DMA lands)

# token chunks per PSUM tile (one matmul output <= 512 fp32 = one bank)
if CAP <= 512:
    CHUNKS = ((0, CAP),)
else:
    CHUNKS = ((0, CAP // 2), (CAP // 2, CAP))

# w1 m1-tile DMA block sizes for m1 = 1..15 (m1=0 travels in `head`):
# fine-grained at the front so early matmuls aren't gated, coarser later.
W1_BLOCKS = (1, 2, 4, 4, 4)

_NC_CACHE = {}


def _round_fp32r(a):
    """Round-to-nearest-even at mantissa bit 12 (fp32r keeps the top 20 bits
    of an fp32 word: 1 sign + 8 exp + 11 explicit mantissa bits)."""
    u = a.view(np.uint32)
    lsb = (u >> 12) & 1
    u = u + 0x7FF + lsb
    u &= np.uint32(0xFFFFF000)
    return u.view(np.float32)


def _build_bass():
    import concourse.bacc as bacc
    import concourse.tile as tile
    from concourse import mybir

    f32 = mybir.dt.float32
    mm_dt = getattr(mybir.dt, MM_DT)

    nc = bacc.Bacc(None, target_bir_lowering=False)
    # head packs the xT k=0 block together with the full w1 m1=0 row-block;
    # its first slice (x k0 + w1 m0k0) unblocks the very first matmul.
    head = nc.declare_dram_parameter("head", [P, CAP + D], mm_dt,
                                     isOutput=False)
    xr = nc.declare_dram_parameter("xr", [(KT1 - 1) * P, CAP], mm_dt,
                                   isOutput=False)
    gb = nc.declare_dram_parameter("gb", [P, MT1], f32, isOutput=False)
    w1r = nc.declare_dram_parameter("w1r", [(MT1 - 1) * P, D], mm_dt,
                                    isOutput=False)
    w2s = nc.declare_dram_parameter("w2s", [F, D], mm_dt, isOutput=False)
    y = nc.declare_dram_parameter("y", [D, CAP], mm_dt, isOutput=True)

    w1_off = []
    off = 1
    for nm in W1_BLOCKS:
        w1_off.append(off)
        off += nm
    assert off == MT1

    with tile.TileContext(nc) as tc:
        with (
            tc.tile_pool(name="wrm", bufs=1) as wrm,
            tc.tile_pool(name="hdp", bufs=1) as hdp,
            tc.tile_pool(name="xp", bufs=1) as xp,
            tc.tile_pool(name="gbp", bufs=1) as gbp,
            tc.tile_pool(name="w1p", bufs=len(W1_BLOCKS)) as w1p,
            tc.tile_pool(name="w2p", bufs=2) as w2p,
            tc.tile_pool(name="hp", bufs=MT1) as hp,
            tc.tile_pool(name="stp", bufs=2) as stp,
            tc.tile_pool(name="psA", bufs=4, space="PSUM") as psA,
            tc.tile_pool(name="psB", bufs=4, space="PSUM") as psB,
        ):
            # PE warm-up: memset a small tile, then issue dummy matmuls that
            # keep the PE busy (heating the HAM clock-gate) while the first
            # real DMA is still in flight.
            garb = wrm.tile([P, 256], mm_dt, tag="garb")
            nc.gpsimd.memset(garb[:], 0)
            for wi in range(N_WARM):
                dummy = psB.tile([P, CHUNKS[0][1] - CHUNKS[0][0]], f32,
                                 tag="psB0", name=f"warm_{wi}")
                nc.tensor.matmul(dummy[:2, :256], garb[:, 0:2], garb[:],
                                 start=True, stop=True, skip_group_check=True)

            # DMA layout across the two HWDGE queues.  All queues stripe the
            # same 16 DMA engines, so splitting BIG streams across queues
            # only hurts (arbitration breaks sequential bursts and lets w2
            # overtake the urgent w1 stream).  Instead:
            #   sync:   head_a, w1 blocks, w2 halves, y outs — exactly the
            #           order phase A/B consume them (FIFO per queue)
            #   scalar: the small early tiles (head_b, x, gb) whose
            #           descriptor generation (~3ns/row, serial per engine)
            #           would otherwise delay the w1 stream behind them
            HA = CAP + P
            head_a = hdp.tile([P, HA], mm_dt, tag="ha")
            nc.sync.dma_start(out=head_a[:], in_=head[:, 0:HA])
            head_b = hdp.tile([P, D - P], mm_dt, tag="hb")
            nc.scalar.dma_start(out=head_b[:], in_=head[:, HA:])

            XSPLIT = 4  # k = 1..3 with the x-lo DMA, 4..7 with x-hi
            x_sb = []
            for k0, k1 in ((1, XSPLIT), (XSPLIT, KT1)):
                t = xp.tile([P, k1 - k0, CAP], mm_dt, tag=f"x{k0}",
                            name=f"x_{k0}")
                src = xr[(k0 - 1) * P:(k1 - 1) * P, :].rearrange(
                    "(j p) c -> p j c", p=P)
                nc.scalar.dma_start(out=t[:], in_=src)
                x_sb.append((k0, t))
            gb_sb = gbp.tile([P, MT1], f32)
            nc.scalar.dma_start(out=gb_sb[:], in_=gb[:, :])

            w1_blk = []
            for j, nm in enumerate(W1_BLOCKS):
                t = w1p.tile([P, nm, D], mm_dt, tag="w1", name=f"w1_{j}",
                             padded_shape=[P, max(W1_BLOCKS), D])
                r0 = (w1_off[j] - 1) * P
                src = w1r[r0:r0 + nm * P, :].rearrange(
                    "(j p) d -> p j d", p=P)
                nc.sync.dma_start(out=t[:], in_=src)
                w1_blk.append(t)

            w2_sb = []
            for j in range(2):
                t = w2p.tile([P, MT1 // 2, D], mm_dt, tag="w2",
                             name=f"w2_{j}")
                src = w2s[j * (F // 2):(j + 1) * (F // 2), :].rearrange(
                    "(j p) d -> p j d", p=P)
                nc.sync.dma_start(out=t[:], in_=src)
                w2_sb.append(t)

            def x_rhs(k, a, b):
                if k == 0:
                    return head_a[:, a:b]
                for k0, t in reversed(x_sb):
                    if k >= k0:
                        return t[:, k - k0, a:b]
                raise AssertionError

            def w1_lhs(m1, k):
                if m1 == 0:
                    if k == 0:
                        return head_a[:, CAP:CAP + P]
                    return head_b[:, (k - 1) * P:k * P]
                j = next(i for i in range(len(W1_BLOCKS))
                         if w1_off[i] <= m1 < w1_off[i] + W1_BLOCKS[i])
                return w1_blk[j][:, m1 - w1_off[j], k * P:(k + 1) * P]

            def primer(name, lhs1, rhs1):
                # borrows an (idle until phase B) psB slot; touches a fresh
                # weight block on PE so later matmuls stay single-wait
                dummy = psB.tile([P, CHUNKS[0][1] - CHUNKS[0][0]], f32,
                                 tag="psB0", name=f"prime_{name}")
                nc.tensor.matmul(dummy[:2, :256], lhs1, rhs1,
                                 start=True, stop=True, skip_group_check=True)

            # Phase A: h^T[F, CAP] = gelu(w1^T @ x^T + b1)
            gelu = mybir.ActivationFunctionType.Gelu
            h_sb = []
            for m1 in range(MT1):
                if m1 > 0 and m1 in w1_off:
                    blk = w1_blk[w1_off.index(m1)]
                    primer(f"w1_{m1}", blk[:, 0, 0:2], blk[:, 0, 0:256])
                pss = [psA.tile([P, b - a], f32, tag=f"psA{i}",
                                name=f"psA{i}_{m1}")
                       for i, (a, b) in enumerate(CHUNKS)]
                for k in range(KT1):
                    lhs = w1_lhs(m1, k)
                    st, sp = (k == 0), (k == KT1 - 1)
                    for i, (a, b) in enumerate(CHUNKS):
                        nc.tensor.matmul(pss[i][:], lhs, x_rhs(k, a, b),
                                         start=st, stop=sp)
                h = hp.tile([P, CAP], mm_dt, tag="h", name=f"h_{m1}")
                bias = gb_sb[:, m1:m1 + 1]
                for i, (a, b) in enumerate(CHUNKS):
                    nc.scalar.activation(h[:, a:b], pss[i][:], gelu, bias=bias)
                h_sb.append(h)

            # Phase B: y^T[D, CAP]; w2 (native [F, D] layout) is the
            # stationary operand, h^T streams.  Output partition dim = D has
            # no padding; the gate moves to the host-side combine.
            # Output pairs (m2 0..5) ship two tiles per DMA; the last two m2
            # ship singly so the critical tail transfer is small.
            for j in range(2):
                primer(f"w2_{j}", w2_sb[j][:, 0, 0:2], w2_sb[j][:, 0, 0:256])
            cp = mybir.ActivationFunctionType.Copy
            stage = None
            for m2 in range(MT2):
                pss = [psB.tile([P, b - a], f32, tag=f"psB{i}",
                                name=f"psB{i}_{m2}")
                       for i, (a, b) in enumerate(CHUNKS)]
                for k2 in range(MT1):
                    lhs = w2_sb[k2 // (MT1 // 2)][:, k2 % (MT1 // 2),
                                                  m2 * P:(m2 + 1) * P]
                    st, sp = (k2 == 0), (k2 == MT1 - 1)
                    for i, (a, b) in enumerate(CHUNKS):
                        nc.tensor.matmul(pss[i][:], lhs,
                                         h_sb[k2][:, a:b], start=st, stop=sp)
                def copy_ps(dst, src, use_vec):
                    if use_vec:
                        nc.vector.tensor_scalar_add(dst, src, 0.0)
                    else:
                        nc.scalar.activation(dst, src, cp)

                if m2 < MT2 - 2:
                    half = m2 % 2
                    if half == 0:
                        stage = stp.tile([P, 2, CAP], mm_dt, tag="stage",
                                         name=f"stage_{m2 // 2}")
                    for i, (a, b) in enumerate(CHUNKS):
                        copy_ps(stage[:, half, a:b], pss[i][:], m2 % 2 == 1)
                    if half == 1:
                        dst = y[(m2 - 1) * P:(m2 + 1) * P, :].rearrange(
                            "(j p) c -> p j c", p=P)
                        nc.sync.dma_start(out=dst, in_=stage[:])
                else:
                    # last two tiles: copy split across scalar+vector, then a
                    # small single-tile DMA each, to shorten the drain tail
                    st1 = stp.tile([P, 1, CAP], mm_dt, tag="stage",
                                   name=f"stage_s{m2}")
                    mid = CAP // 2
                    nc.scalar.activation(st1[:, 0, 0:mid],
                                         pss[0][:, 0:mid], cp)
                    nc.vector.tensor_scalar_add(st1[:, 0, mid:CAP],
                                                pss[-1][:, mid - CHUNKS[-1][0]:],
                                                0.0)
                    nc.sync.dma_start(out=y[m2 * P:(m2 + 1) * P, :],
                                      in_=st1[:, 0, :])
    if not nc.is_finalized():
        nc.finalize()
    return nc


def _get_nc():
    if "nc" not in _NC_CACHE:
        _NC_CACHE["nc"] = _build_bass()
    return _NC_CACHE["nc"]


def kernel(x, dispatch_tensor, combine_tensor, w1, b1, w2, b2, **_):
    import ml_dtypes
    from concourse.bass_utils import run_bass_kernel_spmd

    mm_np = ml_dtypes.bfloat16 if MM_DT == "bfloat16" else np.float32
    rnd = (lambda a: a.astype(mm_np)) if MM_DT == "bfloat16" else _round_fp32r

    x = np.ascontiguousarray(np.asarray(x, dtype=np.float32)).reshape(T, D)
    dispatch = np.asarray(dispatch_tensor, dtype=np.float32).reshape(T, E)
    combine = np.asarray(combine_tensor, dtype=np.float32).reshape(T, E)
    w1 = np.asarray(w1, dtype=np.float32)
    b1 = np.asarray(b1, dtype=np.float32)
    w2 = np.asarray(w2, dtype=np.float32)
    b2 = np.asarray(b2, dtype=np.float32)

    top = dispatch.argmax(-1)
    gate = combine.sum(-1)
    full = [np.nonzero(top == e)[0] for e in range(E)]
    idxs = [idx[:CAP] for idx in full]
    spill = [idx[CAP:] for idx in full]  # over-capacity tokens -> host path

    in_maps = []
    for e in range(E):
        idx = idxs[e]
        c = len(idx)
        xT = np.zeros((D, CAP), np.float32)
        xT[:, :c] = x[idx].T
        # w1s[m1*P+p, k*P+m] = w1[k*P+p, m1*P+m]: per-m1 [P, D] row blocks
        # whose [:, k*P:(k+1)*P] slice is the lhsT k-tile for output tile m1.
        w1s = np.ascontiguousarray(
            w1[e].reshape(KT1, P, MT1, P).transpose(2, 1, 0, 3)
        ).reshape(F, D)
        gbm = np.ascontiguousarray(b1[e].reshape(MT1, P).T)
        xTr, w1sr = rnd(xT), rnd(w1s)
        in_maps.append({
            "head": np.ascontiguousarray(
                np.concatenate([xTr[:P], w1sr[:P]], axis=1)),
            "xr": np.ascontiguousarray(xTr[P:]),
            "gb": gbm,
            "w1r": np.ascontiguousarray(w1sr[P:]),
            "w2s": rnd(np.ascontiguousarray(w2[e])),
        })

    global _LAST_IN_MAPS
    _LAST_IN_MAPS = in_maps
    nc = _get_nc()
    res = run_bass_kernel_spmd(nc, in_maps, list(range(E)))

    y_flat = np.empty((T, D), np.float32)
    for e in range(E):
        idx = idxs[e]
        c = len(idx)
        yT = np.asarray(res.results[e]["y"]).astype(np.float32)  # [D, CAP]
        y_flat[idx] = yT[:, :c].T * gate[idx][:, None]
        if len(spill[e]):
            # capacity-overflow fallback (exact fp32 math on host)
            import math

            erf = np.frompyfunc(math.erf, 1, 1)
            hs = x[spill[e]] @ w1[e] + b1[e]
            hs = hs * 0.5 * (1.0 + erf(hs / np.sqrt(2.0)).astype(np.float64))
            y_flat[spill[e]] = (hs.astype(np.float32) @ w2[e]) * \
                gate[spill[e]][:, None]
    return (y_flat + b2[None, :]).reshape(B, N, D)
